# revision 1
# baseline (speedup 1.0000x reference)
"""NTN kernel, fp8-residual variant: x = bf16(x) + 2^-10 * e4m3(residual*2^10).

Input bytes per element drop from 4 (bf16 hi/lo) to 3 (bf16 + fp8),
cutting the HBM stream from 32 MB to 24 MB per core.

fp8 exponent range forces the residual operands to carry scale:
xl8 = e4m3(r * 2^10), mt8 = e4m3(M^T * 2^3), so their product carries
2^13. Instead of rescaling it, the bf16 weights are pre-scaled by 2^13
(exact: power-of-two shifts of already-rounded bf16 values), so ALL
three matmuls accumulate z * 2^13 into one PSUM bank. The scale unwinds
through the linear tail: negc carries -c * 2^13 and ub carries u * 2^-13.

    y[r] = sum_k (u[k] 2^-13) * max(z[r,k] 2^13, -c[k] 2^13)  + sum_k u[k]c[k]

Engines stay single-duty (critical: a dma_start blocked on a tile slot
stalls every later instruction on its engine):
    SP:   xh DMA issue        ACT:  xl8 DMA issue
    PE:   3 matmuls/tile      DVE:  max + reduce
    GPS:  mul by ub (+ tiny param DMAs at start)

Measured error ~7e-5 L2-relative vs the fp32 reference (bf16 hi/lo
variant: 5.4e-6; fp32 envelope: ~2e-7).
"""

import numpy as np
import ml_dtypes

import concourse.bass as bass
import concourse.bacc as bacc
import concourse.mybir as mybir
import concourse.tile as tile

N, D, K = 500000, 128, 16
NCORES = 8
ROWS_PER_CORE = N // NCORES
TILES = 489
RPC = TILES * 128
GROUP = 32
DMA_CHUNK = 64
F32 = mybir.dt.float32
BF16 = mybir.dt.bfloat16
FP8 = mybir.dt.float8e4
BF = ml_dtypes.bfloat16
E4 = ml_dtypes.float8_e4m3
S1 = 10          # scale on the fp8 residual
S2 = 3           # scale on the fp8 M
SS = 2.0 ** (S1 + S2)


def _build_program():
    nc = bacc.Bacc(None, target_bir_lowering=False)

    xh = nc.dram_tensor("xh", [128, RPC], BF16, kind="ExternalInput")
    xl = nc.dram_tensor("xl", [128, RPC], FP8, kind="ExternalInput")
    mth = nc.dram_tensor("mth", [128, K], BF16, kind="ExternalInput")
    mtl = nc.dram_tensor("mtl", [128, K], BF16, kind="ExternalInput")
    mt8 = nc.dram_tensor("mt8", [128, K], FP8, kind="ExternalInput")
    negc = nc.dram_tensor("negc", [128, GROUP, K], F32, kind="ExternalInput")
    ub = nc.dram_tensor("ub", [128, GROUP, K], F32, kind="ExternalInput")
    y = nc.dram_tensor("y", [128, TILES], F32, kind="ExternalOutput")

    with tile.TileContext(nc) as tc:
        with (
            tc.tile_pool(name="singles", bufs=1) as singles,
            tc.tile_pool(name="xin", bufs=5) as xin,
            tc.tile_pool(name="zp", bufs=6, space="PSUM") as zpool,
            tc.tile_pool(name="work", bufs=4) as work,
            tc.tile_pool(name="yout", bufs=1) as yout,
        ):
            # Uniform big chunks; the final partial chunk is broken into
            # small pieces so that after the last byte lands only a small
            # group's matmuls + reduce remain on the critical path.
            sizes = []
            rem = TILES
            while rem > DMA_CHUNK:
                sizes.append(DMA_CHUNK)
                rem -= DMA_CHUNK
            while rem > 16:
                sizes.append(16)
                rem -= 16
            sizes.append(rem)

            # xh is 2/3 of the bytes, xl8 1/3: greedily assign each chunk's
            # xh to the HWDGE queue with fewer accumulated bytes (xl to the
            # other) so both queues drain together.
            qbytes = {0: 0, 1: 0}   # 0 = sync, 1 = scalar
            engs = (nc.sync, nc.scalar)
            chunk_tiles = []
            c0 = 0
            for nct in sizes:
                xh_t = xin.tile([128, DMA_CHUNK * 128], BF16, tag="xh")
                xl_t = xin.tile([128, DMA_CHUNK * 128], FP8, tag="xl")
                qa = 0 if qbytes[0] <= qbytes[1] else 1
                qb = 1 - qa
                qbytes[qa] += nct * 2
                qbytes[qb] += nct
                engs[qa].dma_start(
                    xh_t[:, : nct * 128], xh[:, c0 * 128 : (c0 + nct) * 128]
                )
                engs[qb].dma_start(
                    xl_t[:, : nct * 128], xl[:, c0 * 128 : (c0 + nct) * 128]
                )
                chunk_tiles.append((c0, nct, xh_t, xl_t))
                c0 += nct
            assert c0 == TILES

            mth_sb = singles.tile([128, K], BF16)
            nc.gpsimd.dma_start(mth_sb, mth[:, :])
            mtl_sb = singles.tile([128, K], BF16)
            nc.gpsimd.dma_start(mtl_sb, mtl[:, :])
            mt8_sb = singles.tile([128, K], FP8)
            nc.gpsimd.dma_start(mt8_sb, mt8[:, :])
            negc_sb = singles.tile([128, GROUP, K], F32)
            nc.gpsimd.dma_start(negc_sb, negc[:, :, :])
            ub_sb = singles.tile([128, GROUP, K], F32)
            nc.gpsimd.dma_start(ub_sb, ub[:, :, :])

            y_sb = yout.tile([128, TILES], F32)

            for c0, nct, xh_t, xl_t in chunk_tiles:
                g0 = 0
                while g0 < nct:
                    nt = min(GROUP, nct - g0)
                    t0 = c0 + g0
                    zp = zpool.tile([128, GROUP, K], F32, tag="z")
                    for t in range(nt):
                        sl = slice((g0 + t) * 128, (g0 + t + 1) * 128)
                        first = t == 0
                        last = t == nt - 1
                        nc.tensor.matmul(
                            zp[:, t, :], xh_t[:, sl], mth_sb[:, :],
                            start=first, stop=False,
                        )
                        nc.tensor.matmul(
                            zp[:, t, :], xh_t[:, sl], mtl_sb[:, :],
                            start=False, stop=False,
                        )
                        nc.tensor.matmul(
                            zp[:, t, :], xl_t[:, sl], mt8_sb[:, :],
                            start=False, stop=last,
                        )
                    relu = work.tile([128, GROUP, K], F32, tag="relu")
                    nc.vector.tensor_tensor(
                        relu[:, :nt, :], zp[:, :nt, :], negc_sb[:, :nt, :],
                        op=mybir.AluOpType.max,
                    )
                    prod = work.tile([128, GROUP, K], F32, tag="prod")
                    nc.gpsimd.tensor_tensor(
                        prod[:, :nt, :], relu[:, :nt, :], ub_sb[:, :nt, :],
                        op=mybir.AluOpType.mult,
                    )
                    nc.vector.tensor_reduce(
                        y_sb[:, t0 : t0 + nt], prod[:, :nt, :],
                        axis=mybir.AxisListType.X, op=mybir.AluOpType.add,
                    )
                    g0 += nt

            yeng = engs[0] if qbytes[0] <= qbytes[1] else engs[1]
            ysplit = TILES - sizes[-1]
            yeng.dma_start(y[:, :ysplit], y_sb[:, :ysplit])
            if ysplit < TILES:
                yeng.dma_start(y[:, ysplit:], y_sb[:, ysplit:])

    nc.compile()
    return nc


_NC_CACHE = None


def _get_program():
    global _NC_CACHE
    if _NC_CACHE is None:
        _NC_CACHE = _build_program()
    return _NC_CACHE


def _host_prep(x1, x2, V, W, b, U):
    x1 = np.asarray(x1, dtype=np.float32)
    x2 = np.asarray(x2, dtype=np.float64)
    V = np.asarray(V, dtype=np.float64)
    W = np.asarray(W, dtype=np.float64)
    b = np.asarray(b, dtype=np.float64)
    U = np.asarray(U, dtype=np.float64)

    M = V[:, :D] + np.einsum("kde,e->kd", W, x2[0])
    c = (x2[0] @ V[:, D:].T) + b
    u = U[:, 0]
    const = float(np.dot(u, c))

    Mh = M.astype(BF)
    Ml = (M - Mh.astype(np.float64)).astype(BF)
    M8 = (M * 2.0**S2).astype(E4)
    # power-of-two scaling of bf16 values is exact (exponent shift)
    mth = np.ascontiguousarray((Mh.astype(np.float64) * SS).astype(BF).T)
    mtl = np.ascontiguousarray((Ml.astype(np.float64) * SS).astype(BF).T)
    mt8 = np.ascontiguousarray(M8.T)
    negc_t = np.broadcast_to(
        (-c * SS).astype(np.float32), (128, GROUP, K)
    ).copy()
    ub_t = np.broadcast_to(
        (u / SS).astype(np.float32), (128, GROUP, K)
    ).copy()

    in_maps = []
    for cidx in range(NCORES):
        sl = x1[cidx * ROWS_PER_CORE : (cidx + 1) * ROWS_PER_CORE]
        slt = sl.T
        hi = slt.astype(BF)
        lo8 = ((slt - hi.astype(np.float32)) * 2.0**S1).astype(E4)
        hbuf = np.zeros((128, RPC), dtype=BF)
        hbuf[:, :ROWS_PER_CORE] = hi
        lbuf = np.zeros((128, RPC), dtype=E4)
        lbuf[:, :ROWS_PER_CORE] = lo8
        in_maps.append(
            {"xh": hbuf, "xl": lbuf, "mth": mth, "mtl": mtl, "mt8": mt8,
             "negc": negc_t, "ub": ub_t}
        )
    return in_maps, const


def _gather(results, const):
    outs = []
    for cidx in range(NCORES):
        yc = np.asarray(results[cidx]["y"])
        outs.append(yc.T.reshape(-1)[:ROWS_PER_CORE])
    yfull = np.concatenate(outs) + np.float32(const)
    return yfull.reshape(N, 1).astype(np.float32)


def run_device(in_maps, trace=False):
    from concourse.bass_utils import run_bass_kernel_spmd

    nc = _get_program()
    res = run_bass_kernel_spmd(
        nc, in_maps, core_ids=list(range(NCORES)), trace=trace
    )
    return res


def kernel(x1, x2, V, W, b, U):
    in_maps, const = _host_prep(x1, x2, V, W, b, U)
    res = run_device(in_maps, trace=False)
    return _gather(res.results, const)



# revision 2
# speedup vs baseline: 1.5968x; 1.5968x over previous
"""NTN kernel, bf16-stream variant.

y = relu(x1 @ M + c) @ u + 0,  M = V[:,:D] + (W @ x2)^T  (128x16),
c = x2 @ V[:,D:]^T + b,        u = U[:,0].

relu(z+c) = max(z,-c) + c turns the affine tail into
    y[r] = sum_k u[k]*max(z[r,k], -c[k]) + sum_k u[k]*c[k]
so the kernel streams x1 (bf16, 2 B/elem -> 16 MB/core), does ONE
128x128 @ 128x16 bf16 matmul per row-tile, then per 32-tile group:
DVE max, Pool mult-by-u, DVE reduce over K.  The fp8-residual variant
(3 B/elem) was DMA-bound at ~99us; pure bf16 cuts the stream 33% and
drops PE work from 3 (ldweights+matmul) pairs per tile to 1, which
removes the PE-issue tail (42% of matmuls used to run after the last
DMA byte landed).

Engines stay single-duty (a dma_start blocked on a tile slot stalls
every later instruction on its engine):
    SP/ACT: x DMA issue (alternating chunks)   PE: 1 matmul/tile
    DVE:    max + reduce                       Pool: mul by u (+ param DMAs)

Measured error ~2.4e-3 L2-relative vs the fp32 reference (x and M both
bf16-rounded; tolerance 2e-2).
"""

import numpy as np
import ml_dtypes

import concourse.bass as bass
import concourse.bacc as bacc
import concourse.mybir as mybir
import concourse.tile as tile

N, D, K = 500000, 128, 16
NCORES = 8
ROWS_PER_CORE = N // NCORES
TILES = 489
RPC = TILES * 128
GROUP = 32
DMA_CHUNK = 64
F32 = mybir.dt.float32
BF16 = mybir.dt.bfloat16
BF = ml_dtypes.bfloat16


def _build_program():
    nc = bacc.Bacc(None, target_bir_lowering=False)

    xh = nc.dram_tensor("xh", [128, RPC], BF16, kind="ExternalInput")
    mt = nc.dram_tensor("mt", [128, K], BF16, kind="ExternalInput")
    negc = nc.dram_tensor("negc", [128, GROUP, K], F32, kind="ExternalInput")
    ub = nc.dram_tensor("ub", [128, GROUP, K], F32, kind="ExternalInput")
    y = nc.dram_tensor("y", [128, TILES], F32, kind="ExternalOutput")

    with tile.TileContext(nc) as tc:
        with (
            tc.tile_pool(name="singles", bufs=1) as singles,
            tc.tile_pool(name="xin", bufs=5) as xin,
            tc.tile_pool(name="zp", bufs=6, space="PSUM") as zpool,
            tc.tile_pool(name="work", bufs=4) as work,
            tc.tile_pool(name="yout", bufs=1) as yout,
        ):
            # Uniform big chunks; the final partial chunk is broken into
            # small pieces so that after the last byte lands only a small
            # group's matmuls + reduce remain on the critical path.
            sizes = []
            rem = TILES
            while rem > DMA_CHUNK:
                sizes.append(DMA_CHUNK)
                rem -= DMA_CHUNK
            while rem > 16:
                sizes.append(16)
                rem -= 16
            sizes.append(rem)

            # Alternate chunks between the two HWDGE queues (sync, scalar)
            # so both drain together.
            engs = (nc.sync, nc.scalar)
            chunk_tiles = []
            c0 = 0
            for i, nct in enumerate(sizes):
                xh_t = xin.tile([128, DMA_CHUNK * 128], BF16, tag="xh")
                engs[i % 2].dma_start(
                    xh_t[:, : nct * 128], xh[:, c0 * 128 : (c0 + nct) * 128]
                )
                chunk_tiles.append((c0, nct, xh_t))
                c0 += nct
            assert c0 == TILES

            mt_sb = singles.tile([128, K], BF16)
            nc.gpsimd.dma_start(mt_sb, mt[:, :])
            negc_sb = singles.tile([128, GROUP, K], F32)
            nc.gpsimd.dma_start(negc_sb, negc[:, :, :])
            ub_sb = singles.tile([128, GROUP, K], F32)
            nc.gpsimd.dma_start(ub_sb, ub[:, :, :])

            y_sb = yout.tile([128, TILES], F32)

            for c0, nct, xh_t in chunk_tiles:
                g0 = 0
                while g0 < nct:
                    nt = min(GROUP, nct - g0)
                    t0 = c0 + g0
                    zp = zpool.tile([128, GROUP, K], F32, tag="z")
                    for t in range(nt):
                        sl = slice((g0 + t) * 128, (g0 + t + 1) * 128)
                        nc.tensor.matmul(
                            zp[:, t, :], xh_t[:, sl], mt_sb[:, :],
                            start=True, stop=True,
                        )
                    relu = work.tile([128, GROUP, K], F32, tag="relu")
                    nc.vector.tensor_tensor(
                        relu[:, :nt, :], zp[:, :nt, :], negc_sb[:, :nt, :],
                        op=mybir.AluOpType.max,
                    )
                    prod = work.tile([128, GROUP, K], F32, tag="prod")
                    nc.gpsimd.tensor_tensor(
                        prod[:, :nt, :], relu[:, :nt, :], ub_sb[:, :nt, :],
                        op=mybir.AluOpType.mult,
                    )
                    nc.vector.tensor_reduce(
                        y_sb[:, t0 : t0 + nt], prod[:, :nt, :],
                        axis=mybir.AxisListType.X, op=mybir.AluOpType.add,
                    )
                    g0 += nt

            ysplit = TILES - sizes[-1]
            nc.sync.dma_start(y[:, :ysplit], y_sb[:, :ysplit])
            if ysplit < TILES:
                nc.sync.dma_start(y[:, ysplit:], y_sb[:, ysplit:])

    nc.compile()
    return nc


_NC_CACHE = None


def _get_program():
    global _NC_CACHE
    if _NC_CACHE is None:
        _NC_CACHE = _build_program()
    return _NC_CACHE


def _host_prep(x1, x2, V, W, b, U):
    x1 = np.asarray(x1, dtype=np.float32)
    x2 = np.asarray(x2, dtype=np.float64)
    V = np.asarray(V, dtype=np.float64)
    W = np.asarray(W, dtype=np.float64)
    b = np.asarray(b, dtype=np.float64)
    U = np.asarray(U, dtype=np.float64)

    M = V[:, :D] + np.einsum("kde,e->kd", W, x2[0])
    c = (x2[0] @ V[:, D:].T) + b
    u = U[:, 0]
    const = float(np.dot(u, c))

    mt = np.ascontiguousarray(M.astype(BF).T)
    negc_t = np.broadcast_to(
        (-c).astype(np.float32), (128, GROUP, K)
    ).copy()
    ub_t = np.broadcast_to(u.astype(np.float32), (128, GROUP, K)).copy()

    in_maps = []
    for cidx in range(NCORES):
        sl = x1[cidx * ROWS_PER_CORE : (cidx + 1) * ROWS_PER_CORE]
        hbuf = np.zeros((128, RPC), dtype=BF)
        hbuf[:, :ROWS_PER_CORE] = sl.T.astype(BF)
        in_maps.append({"xh": hbuf, "mt": mt, "negc": negc_t, "ub": ub_t})
    return in_maps, const


def _gather(results, const):
    outs = []
    for cidx in range(NCORES):
        yc = np.asarray(results[cidx]["y"])
        outs.append(yc.T.reshape(-1)[:ROWS_PER_CORE])
    yfull = np.concatenate(outs) + np.float32(const)
    return yfull.reshape(N, 1).astype(np.float32)


def run_device(in_maps, trace=False):
    from concourse.bass_utils import run_bass_kernel_spmd

    nc = _get_program()
    res = run_bass_kernel_spmd(
        nc, in_maps, core_ids=list(range(NCORES)), trace=trace
    )
    return res


def kernel(x1, x2, V, W, b, U):
    in_maps, const = _host_prep(x1, x2, V, W, b, U)
    res = run_device(in_maps, trace=False)
    return _gather(res.results, const)
